# revision 1
# baseline (speedup 1.0000x reference)
"""Trainium2 Bass kernel for nn_Decoder: attention-GRU decoder.

Key math restructuring (validated vs reference in fp32 and bf16):
 - The attention energy is (h @ vh + vb)[:, None] + enc_score: the h-dependent
   part is constant per row, so softmax over L cancels it. The attention
   weights a = softmax(enc_score) and context ctx are therefore CONSTANT
   across all 512 decode steps and computed once.
 - dec_t is a softmax output; we carry unnormalized E = exp(preact) and its
   row-sum S, normalizing once per step (batch-major, per-partition scalars).
 - The final output out_t = softmax(dec_{t+1} @ Wf.T + bf) is not part of the
   recurrence: dec history is stored and the whole output phase runs batched
   after the loop.

Distribution: pure data parallelism over batch (8 batches per core, zero
cross-core communication). Weights live SBUF-resident in bf16 and stream
through the PE as the moving operand; recurrent state is kept both
batch-major (fp32, for elementwise) and gate-major/transposed (bf16, as the
stationary matmul operand), with PE-transposes bridging the two per step.
"""

import numpy as np
import ml_dtypes

H = 1024
V = 1024
B = 64
L = 512
NCORES = 8
BL = B // NCORES      # batches per core
J = H // 128          # h-chunks

BF16 = ml_dtypes.bfloat16


def _bf(x):
    return np.ascontiguousarray(np.asarray(x, dtype=np.float32)).astype(BF16)


def build_nc(n_steps=L):
    import concourse.bass as bass
    import concourse.tile as tile
    from concourse import bacc, mybir

    f32 = mybir.dt.float32
    bf16 = mybir.dt.bfloat16
    AF = mybir.ActivationFunctionType
    X = mybir.AxisListType.X

    nc = bacc.Bacc()

    # ---- DRAM I/O ----
    enc_n = nc.dram_tensor("enc_n", [L, BL, H], f32, kind="ExternalInput")
    enc_t = nc.dram_tensor("enc_t", [H, BL, L], f32, kind="ExternalInput")
    h0T = nc.dram_tensor("h0T", [128, J * BL], bf16, kind="ExternalInput")
    e0T = nc.dram_tensor("e0T", [128, J * BL], bf16, kind="ExternalInput")
    h0bm = nc.dram_tensor("h0bm", [BL, H], f32, kind="ExternalInput")
    veT_d = nc.dram_tensor("veT", [128, J], f32, kind="ExternalInput")
    wdT_rz = nc.dram_tensor("wdT_rz", [H, 2 * H], bf16, kind="ExternalInput")
    wdT_n = nc.dram_tensor("wdT_n", [H, H], bf16, kind="ExternalInput")
    whT_rz = nc.dram_tensor("whT_rz", [H, 2 * H], bf16, kind="ExternalInput")
    whT_n = nc.dram_tensor("whT_n", [H, H], bf16, kind="ExternalInput")
    woT_d = nc.dram_tensor("woT", [H, 2 * H], bf16, kind="ExternalInput")
    wfT_d = nc.dram_tensor("wfT", [H, V], bf16, kind="ExternalInput")
    wcT_rz = nc.dram_tensor("wcT_rz", [H, 2 * H], bf16, kind="ExternalInput")
    wcT_n = nc.dram_tensor("wcT_n", [H, H], bf16, kind="ExternalInput")
    wocT_d = nc.dram_tensor("wocT", [H, H], bf16, kind="ExternalInput")
    brz8 = nc.dram_tensor("brz8", [BL, 2 * H], bf16, kind="ExternalInput")
    bni8 = nc.dram_tensor("bni8", [BL, H], bf16, kind="ExternalInput")
    bnh8 = nc.dram_tensor("bnh8", [BL, H], bf16, kind="ExternalInput")
    bo8 = nc.dram_tensor("bo8", [BL, H], bf16, kind="ExternalInput")
    bf1 = nc.dram_tensor("bf1", [1, V], bf16, kind="ExternalInput")
    ones128 = nc.dram_tensor("ones128", [1, 128], bf16, kind="ExternalInput")
    ident8_d = nc.dram_tensor("ident8", [8, 8], bf16, kind="ExternalInput")
    ident1_d = nc.dram_tensor("ident1", [1, 1], f32, kind="ExternalInput")
    out_d = nc.dram_tensor("out", [n_steps, BL, V], f32, kind="ExternalOutput")

    with tile.TileContext(nc) as tc:
        # ---------- persistent pools ----------
        with tc.tile_pool(name="wp", bufs=1) as wp, \
             tc.tile_pool(name="state", bufs=1) as stp, \
             tc.tile_pool(name="dramp", bufs=1, space="DRAM") as drp:

            ehist = drp.tile([n_steps, 128, J * BL], bf16)

            def load_w(dram, cols):
                t = wp.tile([128, J, cols], bf16, tag=dram.name)
                nc.sync.dma_start(
                    out=t, in_=dram[:, :].rearrange("(j p) c -> p j c", p=128))
                return t

            wdTrz_sb = load_w(wdT_rz, 2 * H)
            wdTn_sb = load_w(wdT_n, H)
            whTrz_sb = load_w(whT_rz, 2 * H)
            whTn_sb = load_w(whT_n, H)
            woT_sb = load_w(woT_d, 2 * H)

            ident8 = wp.tile([8, 8], bf16, tag="id8")
            nc.sync.dma_start(out=ident8, in_=ident8_d[:, :])
            ident1 = wp.tile([1, 1], f32, tag="id1")
            nc.sync.dma_start(out=ident1, in_=ident1_d[:, :])

            g0rz = wp.tile([BL, 2 * H], bf16, tag="g0rz")
            g0ni = wp.tile([BL, H], bf16, tag="g0ni")
            g0nh = wp.tile([BL, H], bf16, tag="g0nh")
            o08 = wp.tile([BL, H], bf16, tag="o08")
            nc.sync.dma_start(out=g0nh, in_=bnh8[:, :])

            # persistent recurrent state
            HT = stp.tile([128, J * BL], bf16, tag="HT")
            ET = stp.tile([128, J * BL], bf16, tag="ET")
            h_bm = stp.tile([BL, H], f32, tag="h_bm")
            nc.sync.dma_start(out=HT, in_=h0T[:, :])
            nc.sync.dma_start(out=ET, in_=e0T[:, :])
            nc.sync.dma_start(out=h_bm, in_=h0bm[:, :])

            # ---------- phase P: attention precompute ----------
            with tc.tile_pool(name="pp", bufs=1) as pp, \
                 tc.tile_pool(name="pps", bufs=1, space="PSUM") as pps:
                veT = pp.tile([128, J], f32, tag="veT")
                nc.sync.dma_start(out=veT, in_=veT_d[:, :])

                ps_ctx = pps.tile([128, J * BL], f32, tag="ps_ctx")

                for b in range(BL):
                    # -- scores: [1, L] = sum_h enc[l,b,h] * ve[h]
                    ps_s = pps.tile([1, L], f32, tag="ps_s")
                    for j in range(J):
                        et = pp.tile([128, L], f32, tag="enc_t", bufs=3)
                        nc.sync.dma_start(
                            out=et, in_=enc_t[j * 128:(j + 1) * 128, b, :])
                        nc.tensor.matmul(ps_s, veT[:, j:j + 1], et,
                                         start=(j == 0), stop=(j == J - 1))
                    # softmax over L (no max subtraction; scores are small)
                    es = pp.tile([1, L], f32, tag="es", bufs=2)
                    ssum = pp.tile([1, 1], f32, tag="ssum", bufs=2)
                    nc.scalar.activation(out=es, in_=ps_s, func=AF.Exp,
                                         accum_out=ssum)
                    rin = pp.tile([1, 1], f32, tag="rin", bufs=2)
                    nc.vector.reciprocal(out=rin, in_=ssum)
                    an = pp.tile([1, L], f32, tag="an", bufs=2)
                    nc.vector.tensor_scalar_mul(out=an, in0=es, scalar1=rin)
                    # transpose a -> [128, 4]
                    ps_a = pps.tile([128, L // 128], f32, tag="ps_a")
                    for lc in range(L // 128):
                        nc.tensor.transpose(
                            ps_a[:, lc:lc + 1],
                            an[0:1, lc * 128:(lc + 1) * 128], ident1)
                    aT = pp.tile([128, L // 128], f32, tag="aT", bufs=2)
                    nc.vector.tensor_copy(out=aT, in_=ps_a)
                    # -- ctx.T columns: ps_ctx[:, j*BL+b] = sum_l a[l] enc[l,b,hj]
                    for lc in range(L // 128):
                        en = pp.tile([128, H], f32, tag="enc_n", bufs=3)
                        nc.sync.dma_start(
                            out=en, in_=enc_n[lc * 128:(lc + 1) * 128, b, :])
                        for j in range(J):
                            nc.tensor.matmul(
                                ps_ctx[:, j * BL + b:j * BL + b + 1],
                                en[:, j * 128:(j + 1) * 128],
                                aT[:, lc:lc + 1],
                                start=(lc == 0), stop=(lc == L // 128 - 1),
                                skip_group_check=True)

                ctxT = pp.tile([128, J * BL], bf16, tag="ctxT")
                nc.vector.tensor_copy(out=ctxT, in_=ps_ctx)

                # -- G0 constants: ctx @ Wc.T (+ biases), ctx @ Wo_c.T + bo
                def g0_mm(wc_dram, bias_dram, cols, out_sb):
                    ps = pps.tile([BL, cols], f32, tag="ps_g")
                    for j in range(J):
                        w = pp.tile([128, cols], bf16, tag="wc", bufs=2)
                        nc.sync.dma_start(
                            out=w, in_=wc_dram[j * 128:(j + 1) * 128, :])
                        for k in range(cols // 512):
                            nc.tensor.matmul(
                                ps[:, k * 512:(k + 1) * 512],
                                ctxT[:, j * BL:(j + 1) * BL],
                                w[:, k * 512:(k + 1) * 512],
                                start=(j == 0), stop=False,
                                skip_group_check=True)
                    bt = pp.tile([BL, cols], bf16, tag="bias", bufs=2)
                    nc.sync.dma_start(out=bt, in_=bias_dram[:, :])
                    for k in range(cols // 512):
                        nc.tensor.matmul(ps[:, k * 512:(k + 1) * 512],
                                         ident8, bt[:, k * 512:(k + 1) * 512],
                                         start=False, stop=True,
                                         skip_group_check=True)
                    nc.vector.tensor_copy(out=out_sb, in_=ps)

                g0_mm(wcT_rz, brz8, 2 * H, g0rz)
                g0_mm(wcT_n, bni8, H, g0ni)
                g0_mm(wocT_d, bo8, H, o08)

            # ---------- phase L: the recurrence ----------
            with tc.tile_pool(name="lp", bufs=2) as lp, \
                 tc.tile_pool(name="lps", bufs=1, space="PSUM") as lps:
                for t in range(n_steps):
                    # -- gate preacts
                    pA = lps.tile([BL, 2 * H], f32, tag="pA")
                    pB = lps.tile([BL, H], f32, tag="pB")
                    pC = lps.tile([BL, H], f32, tag="pC")
                    for k in range(4):
                        sl = slice(k * 512, (k + 1) * 512)
                        for j in range(J):
                            nc.tensor.matmul(
                                pA[:, sl], HT[:, j * BL:(j + 1) * BL],
                                whTrz_sb[:, j, sl],
                                start=(j == 0), stop=False,
                                skip_group_check=True)
                        for j in range(J):
                            nc.tensor.matmul(
                                pA[:, sl], ET[:, j * BL:(j + 1) * BL],
                                wdTrz_sb[:, j, sl],
                                start=False, stop=False, skip_group_check=True)
                        nc.tensor.matmul(pA[:, sl], ident8, g0rz[:, sl],
                                         start=False, stop=True,
                                         skip_group_check=True)
                    for k in range(2):
                        sl = slice(k * 512, (k + 1) * 512)
                        for j in range(J):
                            nc.tensor.matmul(
                                pB[:, sl], ET[:, j * BL:(j + 1) * BL],
                                wdTn_sb[:, j, sl],
                                start=(j == 0), stop=False,
                                skip_group_check=True)
                        nc.tensor.matmul(pB[:, sl], ident8, g0ni[:, sl],
                                         start=False, stop=True,
                                         skip_group_check=True)
                        for j in range(J):
                            nc.tensor.matmul(
                                pC[:, sl], HT[:, j * BL:(j + 1) * BL],
                                whTn_sb[:, j, sl],
                                start=(j == 0), stop=False,
                                skip_group_check=True)
                        nc.tensor.matmul(pC[:, sl], ident8, g0nh[:, sl],
                                         start=False, stop=True,
                                         skip_group_check=True)

                    # -- gates (batch-major, fp32)
                    rz = lp.tile([BL, 2 * H], f32, tag="rz")
                    nc.scalar.activation(out=rz, in_=pA, func=AF.Sigmoid)
                    nh = lp.tile([BL, H], f32, tag="ew", bufs=3)
                    nc.vector.tensor_mul(out=nh, in0=rz[:, :H], in1=pC)
                    npre = lp.tile([BL, H], f32, tag="ew", bufs=3)
                    nc.vector.tensor_add(out=npre, in0=nh, in1=pB)
                    ng = lp.tile([BL, H], f32, tag="ew", bufs=3)
                    nc.scalar.activation(out=ng, in_=npre, func=AF.Tanh)
                    d = lp.tile([BL, H], f32, tag="ew", bufs=3)
                    nc.vector.tensor_sub(out=d, in0=h_bm, in1=ng)
                    zd = lp.tile([BL, H], f32, tag="ew", bufs=3)
                    nc.vector.tensor_mul(out=zd, in0=rz[:, H:], in1=d)
                    nc.vector.tensor_add(out=h_bm, in0=ng, in1=zd)
                    hnb = lp.tile([BL, H], bf16, tag="hnb")
                    nc.vector.tensor_copy(out=hnb, in_=h_bm)

                    # -- transpose h_new into gate-major state
                    pT1 = lps.tile([128, J * BL], bf16, tag="pB")
                    for j in range(J):
                        nc.tensor.transpose(
                            pT1[:, j * BL:(j + 1) * BL],
                            hnb[:, j * 128:(j + 1) * 128], ident8)
                    nc.vector.tensor_copy(out=HT, in_=pT1)

                    # -- dec preactivation
                    pD = lps.tile([BL, H], f32, tag="pA")
                    for k in range(2):
                        sl = slice(k * 512, (k + 1) * 512)
                        for j in range(J):
                            nc.tensor.matmul(
                                pD[:, sl], HT[:, j * BL:(j + 1) * BL],
                                woT_sb[:, j, sl],
                                start=(j == 0), stop=False,
                                skip_group_check=True)
                        for j in range(J):
                            nc.tensor.matmul(
                                pD[:, sl], ET[:, j * BL:(j + 1) * BL],
                                woT_sb[:, j, H + k * 512:H + (k + 1) * 512],
                                start=False, stop=False, skip_group_check=True)
                        nc.tensor.matmul(pD[:, sl], ident8, o08[:, sl],
                                         start=False, stop=True,
                                         skip_group_check=True)

                    # -- E = exp(pD), S = row sums, normalized bf16 dec
                    ebm = lp.tile([BL, H], f32, tag="ebm")
                    s8 = lp.tile([BL, 1], f32, tag="s8")
                    nc.scalar.activation(out=ebm, in_=pD, func=AF.Exp,
                                         accum_out=s8)
                    sinv = lp.tile([BL, 1], f32, tag="sinv")
                    nc.vector.reciprocal(out=sinv, in_=s8)
                    enb = lp.tile([BL, H], bf16, tag="enb")
                    nc.vector.tensor_scalar_mul(out=enb, in0=ebm, scalar1=sinv)

                    # -- transpose dec into gate-major state + store history
                    pT2 = lps.tile([128, J * BL], bf16, tag="pC")
                    for j in range(J):
                        nc.tensor.transpose(
                            pT2[:, j * BL:(j + 1) * BL],
                            enb[:, j * 128:(j + 1) * 128], ident8)
                    nc.vector.tensor_copy(out=ET, in_=pT2)
                    nc.sync.dma_start(out=ehist[t, :, :], in_=ET)

            # ---------- phase F: out[t] = softmax(dec_{t+1} @ Wf.T + bf) ----------
            TB = 128 // BL  # steps per block
            with tc.tile_pool(name="fp", bufs=1) as fp, \
                 tc.tile_pool(name="fps", bufs=2, space="PSUM") as fps:
                wf_sb = fp.tile([128, J, V], bf16, tag="wf")
                nc.sync.dma_start(
                    out=wf_sb, in_=wfT_d[:, :].rearrange("(j p) c -> p j c", p=128))
                ones_sb = fp.tile([1, 128], bf16, tag="ones")
                nc.sync.dma_start(out=ones_sb, in_=ones128[:, :])
                bf_sb = fp.tile([1, V], bf16, tag="bf1")
                nc.sync.dma_start(out=bf_sb, in_=bf1[:, :])

                for t0 in range(0, n_steps, TB):
                    nt = min(TB, n_steps - t0)
                    rows = nt * BL
                    pF = fps.tile([128, V], f32, tag="pF")
                    for j in range(J):
                        lh = fp.tile([128, TB * BL], bf16, tag="lh", bufs=3)
                        nc.sync.dma_start(
                            out=lh[:, :rows],
                            in_=ehist[t0:t0 + nt, :, j * BL:(j + 1) * BL]
                            .transpose([1, 0, 2]))
                        for k in range(V // 512):
                            nc.tensor.matmul(
                                pF[:rows, k * 512:(k + 1) * 512],
                                lh[:, :rows],
                                wf_sb[:, j, k * 512:(k + 1) * 512],
                                start=(j == 0), stop=False,
                                skip_group_check=True)
                    for k in range(V // 512):
                        nc.tensor.matmul(
                            pF[:rows, k * 512:(k + 1) * 512],
                            ones_sb[:, :rows],
                            bf_sb[:, k * 512:(k + 1) * 512],
                            start=False, stop=True, skip_group_check=True)
                    fe = fp.tile([128, V], f32, tag="fe", bufs=2)
                    fs = fp.tile([128, 1], f32, tag="fs", bufs=2)
                    nc.scalar.activation(out=fe[:rows], in_=pF[:rows],
                                         func=AF.Exp, accum_out=fs[:rows])
                    fin = fp.tile([128, 1], f32, tag="fin", bufs=2)
                    nc.vector.reciprocal(out=fin[:rows], in_=fs[:rows])
                    ob = fp.tile([128, V], f32, tag="ob", bufs=2)
                    nc.vector.tensor_scalar_mul(out=ob[:rows], in0=fe[:rows],
                                                scalar1=fin[:rows])
                    nc.sync.dma_start(
                        out=out_d[t0:t0 + nt, :, :].rearrange("t b v -> (t b) v"),
                        in_=ob[:rows])

    nc.compile()
    return nc


def prep_in_maps(encoder_output, hidden, decoder_output, Wa, ba, v,
                 W_ih, b_ih, W_hh, b_hh, Wo, bo, Wf, bf, n_steps=L):
    """Host-side sharding + weight layout transforms (numpy only)."""
    enc = np.asarray(encoder_output, np.float32)
    hid = np.asarray(hidden, np.float32)
    dec = np.asarray(decoder_output, np.float32)
    Wa = np.asarray(Wa, np.float32)
    v = np.asarray(v, np.float32)
    W_ih = np.asarray(W_ih, np.float32)
    W_hh = np.asarray(W_hh, np.float32)
    b_ih = np.asarray(b_ih, np.float32)
    b_hh = np.asarray(b_hh, np.float32)
    Wo = np.asarray(Wo, np.float32)
    bo = np.asarray(bo, np.float32)
    Wf = np.asarray(Wf, np.float32)
    bf = np.asarray(bf, np.float32)

    ve = Wa[:, H:].T @ v                       # (H,)
    veT = np.ascontiguousarray(ve.reshape(J, 128).T)   # [128, J]

    wd = W_ih[:, :H]
    wc = W_ih[:, H:]
    shared = {
        "veT": veT,
        "wdT_rz": _bf(wd[:2 * H].T),
        "wdT_n": _bf(wd[2 * H:].T),
        "whT_rz": _bf(W_hh[:2 * H].T),
        "whT_n": _bf(W_hh[2 * H:].T),
        "woT": _bf(np.concatenate([Wo[:, :H].T, Wo[:, H:2 * H].T], axis=1)),
        "wfT": _bf(Wf.T),
        "wcT_rz": _bf(wc[:2 * H].T),
        "wcT_n": _bf(wc[2 * H:].T),
        "wocT": _bf(Wo[:, 2 * H:].T),
        "brz8": _bf(np.broadcast_to(b_ih[:2 * H] + b_hh[:2 * H], (BL, 2 * H))),
        "bni8": _bf(np.broadcast_to(b_ih[2 * H:], (BL, H))),
        "bnh8": _bf(np.broadcast_to(b_hh[2 * H:], (BL, H))),
        "bo8": _bf(np.broadcast_to(bo, (BL, H))),
        "bf1": _bf(bf.reshape(1, V)),
        "ones128": _bf(np.ones((1, 128))),
        "ident8": _bf(np.eye(8)),
        "ident1": np.ones((1, 1), np.float32),
    }

    def gate_major(x_bm):       # [BL, H] -> [128, J*BL]
        # col j*BL+b, row p  <-  x[b, j*128+p]
        return np.ascontiguousarray(
            x_bm.reshape(BL, J, 128).transpose(2, 1, 0).reshape(128, J * BL))

    in_maps = []
    for c in range(NCORES):
        bs = slice(c * BL, (c + 1) * BL)
        enc_c = enc[:, bs, :]                      # [L, BL, H]
        m = dict(shared)
        m["enc_n"] = np.ascontiguousarray(enc_c)
        m["enc_t"] = np.ascontiguousarray(enc_c.transpose(2, 1, 0))  # [H, BL, L]
        m["h0T"] = gate_major(hid[bs]).astype(BF16)
        m["e0T"] = gate_major(dec[bs]).astype(BF16)
        m["h0bm"] = np.ascontiguousarray(hid[bs])
        in_maps.append(m)
    return in_maps


_NC_CACHE = {}


def get_nc(n_steps=L):
    if n_steps not in _NC_CACHE:
        _NC_CACHE[n_steps] = build_nc(n_steps)
    return _NC_CACHE[n_steps]


def kernel(**inputs):
    from concourse.bass_utils import run_bass_kernel_spmd
    nc = get_nc(L)
    in_maps = prep_in_maps(**inputs)
    res = run_bass_kernel_spmd(nc, in_maps, core_ids=list(range(NCORES)))
    out = np.zeros((L, B, V), np.float32)
    for c in range(NCORES):
        out[:, c * BL:(c + 1) * BL, :] = res.results[c]["out"]
    return out
